# revision 38
# baseline (speedup 1.0000x reference)
"""Trainium2 Bass kernel for nn_GRUNetBinaryEmbeding.

Reference computation (PyTorch GRU semantics, gate order r,z,n; no biases):
    emb = embed[x]                                # [B,T,H]
    per t: gi = emb_t @ w_ih.T ; gh = h @ w_hh.T
           r = sig(gi_r + gh_r); z = sig(gi_z + gh_z)
           n = tanh(gi_n + r * gh_n)
           h = (1-z)*n + z*h
    out  = sigmoid(relu(h_t) @ dec_w.T + dec_b)   # [B,T,1]
    also returns h_last [1,B,H]

Shapes: VOCAB=40, H=256, OUT=1, B=256, T=512.

Strategy (8 NeuronCores, data-parallel over batch, B_local=32/core):
  - key algebraic move: gi = (embed @ w_ih.T)[x] = G[x]; realized on device as
    one-hot(x) matmuls against the precomputed G table [40,768], which
    accumulate directly into the same PSUM tiles as the h @ w_hh.T partial
    products (PE does the gi+gh adds for the r,z gates for free).
  - transposed layout everywhere: [hidden -> partitions, batch -> free], so
    elementwise gate math runs on full 128-lane tiles [128, 64] and the
    recurrent matmuls need no per-step transpose (rhs = h.T chunks).
  - per step: 18 matmuls (6 gate M-chunks x (2 h K-chunks + one-hot), N=32)
    into 2 PSUM banks; sigmoid/tanh on ScalarE; 6 DVE ops for the cell update.
  - decoder (dec_w [1,256]) runs as a post-pass over the relu'd hidden-state
    history kept in SBUF: M=1 matmuls + fused sigmoid(+bias) on ScalarE.
"""

import numpy as np
import ml_dtypes

import concourse.bass as bass
import concourse.tile as tile
from concourse.tile import add_dep_helper
from concourse import bacc
from concourse import mybir
from concourse.bass_utils import run_bass_kernel_spmd

# ---- problem constants (hardcoded per the task contract) ----
VOCAB, H, B, T = 40, 256, 256, 512
NCORES = 8
BL = B // NCORES          # batch per core = 32
HC = 2                    # hidden chunks of 128
G3 = 3 * H                # 768 gate dim
DEC_W = 16                # decoder window (timesteps per decode matmul)

# weight/activation dtype for matmul operands and carried state (PSUM and all
# gate math accumulate in fp32 on-engine; bf16 keeps DVE 2x modes and FWL).
WNP = ml_dtypes.bfloat16
WDT = mybir.dt.bfloat16
F32 = mybir.dt.float32
AF = mybir.ActivationFunctionType
OP = mybir.AluOpType


def _build_bass(T_steps: int = T):
    nc = bacc.Bacc("TRN2", target_bir_lowering=False, debug=False)

    # ---- DRAM I/O ----
    oh_d = nc.dram_tensor("oh", [VOCAB, T_steps * BL], WDT, kind="ExternalInput")
    h0_d = nc.dram_tensor("h0t", [128, HC, BL], F32, kind="ExternalInput")
    wht_d = nc.dram_tensor("wht", [128, HC, G3], WDT, kind="ExternalInput")
    g_d = nc.dram_tensor("g", [VOCAB, G3], WDT, kind="ExternalInput")
    decw_d = nc.dram_tensor("decwt", [128, HC], WDT, kind="ExternalInput")
    decb_d = nc.dram_tensor("decb", [1, 1], F32, kind="ExternalInput")

    n_win = T_steps // DEC_W
    hlast_d = nc.dram_tensor("hlast", [128, HC, BL], WDT, kind="ExternalOutput")
    deco_d = nc.dram_tensor("deco", [n_win, DEC_W * BL], F32, kind="ExternalOutput")

    from contextlib import ExitStack
    with tile.TileContext(nc) as tc, ExitStack() as ctx:
        cpool = ctx.enter_context(tc.tile_pool(name="const", bufs=1))
        spool = ctx.enter_context(tc.tile_pool(name="work", bufs=2))
        hpool = ctx.enter_context(tc.tile_pool(name="hstate", bufs=3))
        ppool = ctx.enter_context(tc.tile_pool(name="psum", bufs=2, space="PSUM"))

        h_cur = hpool.tile([128, HC, BL], F32, tag="h")
        nc.sync.dma_start(h_cur[:], h0_d.ap())
        g_sb = cpool.tile([VOCAB, G3], WDT)
        nc.scalar.dma_start(g_sb[:], g_d.ap())
        wht_sb = cpool.tile([128, HC, G3], WDT)
        nc.scalar.dma_start(wht_sb[:], wht_d.ap())
        oh_sb = cpool.tile([VOCAB, T_steps * BL], WDT)
        n_oh_chunks = max(1, T_steps // 64)
        ohc = T_steps * BL // n_oh_chunks
        for i in range(n_oh_chunks):
            nc.sync.dma_start(oh_sb[:, i * ohc:(i + 1) * ohc],
                              oh_d.ap()[:, i * ohc:(i + 1) * ohc])
        decw_sb = cpool.tile([128, HC], WDT)
        nc.scalar.dma_start(decw_sb[:], decw_d.ap())
        decb_sb = cpool.tile([1, 1], F32)
        nc.scalar.dma_start(decb_sb[:], decb_d.ap())
        # relu(h_t) history, [128, t, kc, b]
        hist = cpool.tile([128, T_steps, HC, BL], WDT)

        h_w = hpool.tile([128, HC, BL], WDT, tag="hw")
        nc.vector.tensor_copy(h_w.rearrange("p a b -> p (a b)"),
                              h_cur.rearrange("p a b -> p (a b)"))

        # psum tiles are allocated one step ahead so the h-independent
        # one-hot (gi) matmuls of step t+1 can run in step t's chain shadow.
        def alloc_step_tiles():
            p_r = ppool.tile([128, 2 * BL], F32, tag="r", name="p_r")
            p_z = ppool.tile([128, 2 * BL], F32, tag="z", name="p_z")
            p_n = ppool.tile([128, 4 * BL], F32, tag="nn", name="p_n")
            return p_r, p_z, p_n

        def emit_gi(tiles, t):
            # start=True only on the FIRST matmul into each psum bank per
            # step-cycle: start clears has_written for the WHOLE bank, so a
            # second start=True would wipe the first region's accumulate bits.
            # Later region-initial matmuls use start=False and overwrite
            # (bits are clear there), while same-region matmuls accumulate.
            p_r, p_z, p_n = tiles
            oh_t = oh_sb[:, t * BL:(t + 1) * BL]            # [40, 32]
            for c in range(2):      # r chunks
                nc.tensor.matmul(p_r[:, c * BL:(c + 1) * BL],
                                 g_sb[:, c * 128:(c + 1) * 128],
                                 oh_t, start=(c == 0), stop=False,
                                 skip_group_check=True)
            for c in range(2):      # z chunks
                nc.tensor.matmul(p_z[:, c * BL:(c + 1) * BL],
                                 g_sb[:, (2 + c) * 128:(3 + c) * 128],
                                 oh_t, start=(c == 0), stop=False,
                                 skip_group_check=True)
            last = None
            for c in range(2):      # n chunks (gi-only region)
                last = nc.tensor.matmul(p_n[:, (2 + c) * BL:(3 + c) * BL],
                                        g_sb[:, (4 + c) * 128:(5 + c) * 128],
                                        oh_t, start=(c == 0), stop=False,
                                        skip_group_check=True)
            return last

        anchor_pe = {}   # step -> last gi matmul inst (PE ordering anchor)
        anchor_act = {}  # step -> tanh inst (ACT ordering anchor)

        def emit_decode(w):
            # out[t,b] = sigmoid(sum_h relu(h)[h,t,b] * dec_w[h] + b)
            # Ordering-only deps pin each piece into a specific step's engine
            # slack so the static scheduler cannot hoist it into the critical
            # FIFO position of an earlier step.
            base = min(w * DEC_W + DEC_W, T_steps - 1)
            HW2 = DEC_W // 2
            pd = ppool.tile([1, DEC_W * BL], F32, tag="dec", name="pd", bufs=1)
            k = 0
            for half in range(2):
                t0, t1 = w * DEC_W + half * HW2, w * DEC_W + (half + 1) * HW2
                dst = pd[:, half * HW2 * BL:(half + 1) * HW2 * BL]
                for kc in range(HC):
                    rhs = hist[:, t0:t1, kc, :]
                    mm = nc.tensor.matmul(dst, decw_sb[:, kc:kc + 1], rhs,
                                          start=(kc == 0), stop=(kc == HC - 1))
                    a = anchor_pe.get(min(base + k, T_steps - 1))
                    if a is not None:
                        add_dep_helper(mm.ins, a.ins, sync=False,
                                       reason="decode mm in PE slack")
                    k += 1
            ds = spool.tile([1, DEC_W * BL], F32, tag="ds")
            for half in range(2):
                sl = slice(half * HW2 * BL, (half + 1) * HW2 * BL)
                sg = nc.scalar.activation(ds[:, sl], pd[:, sl], AF.Sigmoid,
                                          bias=decb_sb[:, 0:1])
                a = anchor_act.get(min(base + 4 + half, T_steps - 1))
                if a is not None:
                    add_dep_helper(sg.ins, a.ins, sync=False,
                                   reason="decode sigmoid after tanh")
            nc.sync.dma_start(deco_d.ap()[w:w + 1, :], ds[:])

        cur_tiles = alloc_step_tiles()
        emit_gi(cur_tiles, 0)

        for t in range(T_steps):
            p_r, p_z, p_n = cur_tiles

            # gh accumulation: r chunks first (gate the chain), then z, then hn
            for c in range(2):
                dst = p_r[:, c * BL:(c + 1) * BL]
                nc.tensor.matmul(dst, wht_sb[:, 0, c * 128:(c + 1) * 128],
                                 h_w[:, 0, :], start=False, stop=False,
                                 skip_group_check=True)
                nc.tensor.matmul(dst, wht_sb[:, 1, c * 128:(c + 1) * 128],
                                 h_w[:, 1, :], start=False, stop=(c == 1),
                                 skip_group_check=True)
            for c in range(2):
                dst = p_z[:, c * BL:(c + 1) * BL]
                nc.tensor.matmul(dst, wht_sb[:, 0, (2 + c) * 128:(3 + c) * 128],
                                 h_w[:, 0, :], start=False, stop=False,
                                 skip_group_check=True)
                nc.tensor.matmul(dst, wht_sb[:, 1, (2 + c) * 128:(3 + c) * 128],
                                 h_w[:, 1, :], start=False, stop=(c == 1),
                                 skip_group_check=True)
            for c in range(2):      # hn: gh only (bank bits cleared by gi_n0)
                dst = p_n[:, c * BL:(c + 1) * BL]
                nc.tensor.matmul(dst, wht_sb[:, 0, (4 + c) * 128:(5 + c) * 128],
                                 h_w[:, 0, :], start=False, stop=False,
                                 skip_group_check=True)
                nc.tensor.matmul(dst, wht_sb[:, 1, (4 + c) * 128:(5 + c) * 128],
                                 h_w[:, 1, :], start=False, stop=(c == 1),
                                 skip_group_check=True)

            # sigmoid split: r first (unblocks t1), z next
            sr = spool.tile([128, 2 * BL], WDT, tag="sr")
            nc.scalar.activation(sr[:], p_r[:], AF.Sigmoid)
            sz = spool.tile([128, 2 * BL], WDT, tag="sz")
            nc.scalar.activation(sz[:], p_z[:], AF.Sigmoid)

            t1 = spool.tile([128, 2 * BL], WDT, tag="t1")     # r * hn
            nc.vector.tensor_tensor(t1[:], sr[:], p_n[:, 0:2 * BL], OP.mult)
            t2 = spool.tile([128, 2 * BL], WDT, tag="t2")     # + gn
            nc.vector.tensor_tensor(t2[:], t1[:], p_n[:, 2 * BL:4 * BL], OP.add)

            # tail operands, both in tanh's shadow:
            sm = spool.tile([128, 2 * BL], WDT, tag="sm")     # 1 - z
            nc.vector.tensor_scalar(sm[:], sz[:], 1.0, -1.0,
                                    OP.subtract, OP.mult)
            hw_flat = h_w.rearrange("p a b -> p (a b)")
            zh = spool.tile([128, 2 * BL], WDT, tag="zh")     # z * h
            nc.vector.tensor_tensor(zh[:], sz[:], hw_flat, OP.mult)

            n_t = spool.tile([128, 2 * BL], WDT, tag="nt")
            anchor_act[t] = nc.scalar.activation(n_t[:], t2[:], AF.Tanh)
            vp = spool.tile([128, 2 * BL], WDT, tag="vp")     # (1-z)*n
            nc.vector.tensor_tensor(vp[:], sm[:], n_t[:], OP.mult)
            hw_new = hpool.tile([128, HC, BL], WDT, tag="hw")
            nc.vector.tensor_tensor(hw_new.rearrange("p a b -> p (a b)"),
                                    zh[:], vp[:], OP.add)
            h_w = hw_new
            # prefetch next step's gi matmuls into this chain's shadow
            if t + 1 < T_steps:
                cur_tiles = alloc_step_tiles()
                anchor_pe[t] = emit_gi(cur_tiles, t + 1)
            # relu'd history for the decoder (DVE slack, off the critical path)
            nc.vector.tensor_scalar_max(hist[:, t, :, :].rearrange("p a b -> p (a b)"),
                                        h_w.rearrange("p a b -> p (a b)"), 0.0)

        nc.sync.dma_start(hlast_d.ap(), h_w[:])

        # decoder post-pass
        for w in range(n_win):
            emit_decode(w)

    nc.compile()
    return nc


def _prep_inputs(x, h0, embed, w_ih, w_hh, dec_w, dec_b, T_steps=T):
    x = np.asarray(x)
    h0 = np.asarray(h0, dtype=np.float32)
    embed = np.asarray(embed, dtype=np.float32)
    w_ih = np.asarray(w_ih, dtype=np.float32)
    w_hh = np.asarray(w_hh, dtype=np.float32)
    dec_w = np.asarray(dec_w, dtype=np.float32)
    dec_b = np.asarray(dec_b, dtype=np.float32)

    G = (embed @ w_ih.T).astype(WNP)                                   # [40, 768]
    wht = np.ascontiguousarray(
        w_hh.T.reshape(HC, 128, G3).transpose(1, 0, 2)).astype(WNP)    # [128,2,768]
    decwt = np.ascontiguousarray(dec_w.reshape(HC, 128).T).astype(WNP)  # [128, 2]
    decb = dec_b.reshape(1, 1)

    in_maps = []
    for c in range(NCORES):
        xc = x[c * BL:(c + 1) * BL, :T_steps]                          # [32, T]
        idx = np.ascontiguousarray(xc.T).reshape(-1)                   # t-major
        oh = (np.arange(VOCAB)[:, None] == idx[None, :]).astype(WNP)
        h0c = h0[0, c * BL:(c + 1) * BL, :]                            # [32, 256]
        h0t = np.ascontiguousarray(
            h0c.T.reshape(HC, 128, BL).transpose(1, 0, 2))             # [128,2,32]
        in_maps.append({
            "oh": oh, "h0t": h0t, "wht": wht, "g": G,
            "decwt": decwt, "decb": decb,
        })
    return in_maps


_NC_CACHE = {}


def kernel(x, h0, embed, w_ih, w_hh, dec_w, dec_b, T_steps=T, want_trace=False):
    if T_steps not in _NC_CACHE:
        _NC_CACHE[T_steps] = _build_bass(T_steps)
    nc = _NC_CACHE[T_steps]
    in_maps = _prep_inputs(x, h0, embed, w_ih, w_hh, dec_w, dec_b, T_steps)
    res = run_bass_kernel_spmd(nc, in_maps, core_ids=list(range(NCORES)),
                               trace=want_trace)

    n_win = T_steps // DEC_W
    outs, hlasts = [], []
    for c in range(NCORES):
        r = res.results[c]
        deco = r["deco"].reshape(n_win, DEC_W, BL).transpose(2, 0, 1)
        outs.append(deco.reshape(BL, T_steps))
        hl = r["hlast"].transpose(2, 1, 0).reshape(BL, H)
        hlasts.append(hl)
    out = np.concatenate(outs, axis=0)[:, :, None].astype(np.float32)  # [B,T,1]
    h_last = np.concatenate(hlasts, axis=0)[None].astype(np.float32)   # [1,B,H]
    if want_trace:
        return (out, h_last), res
    return out, h_last


# revision 39
# speedup vs baseline: 1.2627x; 1.2627x over previous
"""Trainium2 Bass kernel for nn_GRUNetBinaryEmbeding.

Reference computation (PyTorch GRU semantics, gate order r,z,n; no biases):
    emb = embed[x]                                # [B,T,H]
    per t: gi = emb_t @ w_ih.T ; gh = h @ w_hh.T
           r = sig(gi_r + gh_r); z = sig(gi_z + gh_z)
           n = tanh(gi_n + r * gh_n)
           h = (1-z)*n + z*h
    out  = sigmoid(relu(h_t) @ dec_w.T + dec_b)   # [B,T,1]
    also returns h_last [1,B,H]

Shapes: VOCAB=40, H=256, OUT=1, B=256, T=512.

Strategy (8 NeuronCores, data-parallel over batch, B_local=32/core):
  - key algebraic move: gi = (embed @ w_ih.T)[x] = G[x]; realized on device as
    one-hot(x) matmuls against the precomputed G table [40,768], which
    accumulate directly into the same PSUM tiles as the h @ w_hh.T partial
    products (PE does the gi+gh adds for the r,z gates for free).
  - transposed layout everywhere: [hidden -> partitions, batch -> free], so
    elementwise gate math runs on full 128-lane tiles [128, 64] and the
    recurrent matmuls need no per-step transpose (rhs = h.T chunks).
  - per step: 18 matmuls (6 gate M-chunks x (2 h K-chunks + one-hot), N=32)
    into 2 PSUM banks; sigmoid/tanh on ScalarE; 6 DVE ops for the cell update.
  - decoder (dec_w [1,256]) runs as a post-pass over the relu'd hidden-state
    history kept in SBUF: M=1 matmuls + fused sigmoid(+bias) on ScalarE.
"""

import numpy as np
import ml_dtypes

import concourse.bass as bass
import concourse.tile as tile
from concourse.tile import add_dep_helper
from concourse import bacc
from concourse import mybir
from concourse.bass_utils import run_bass_kernel_spmd

# ---- problem constants (hardcoded per the task contract) ----
VOCAB, H, B, T = 40, 256, 256, 512
NCORES = 8
BL = B // NCORES          # batch per core = 32
HC = 2                    # hidden chunks of 128
G3 = 3 * H                # 768 gate dim
DEC_W = 16                # decoder window (timesteps per decode matmul)

# weight/activation dtype for matmul operands and carried state (PSUM and all
# gate math accumulate in fp32 on-engine; bf16 keeps DVE 2x modes and FWL).
WNP = ml_dtypes.bfloat16
WDT = mybir.dt.bfloat16
F32 = mybir.dt.float32
AF = mybir.ActivationFunctionType
OP = mybir.AluOpType


def _build_bass(T_steps: int = T):
    nc = bacc.Bacc("TRN2", target_bir_lowering=False, debug=False)

    # ---- DRAM I/O ----
    oh_d = nc.dram_tensor("oh", [VOCAB, T_steps * BL], WDT, kind="ExternalInput")
    h0_d = nc.dram_tensor("h0t", [128, HC, BL], F32, kind="ExternalInput")
    wht_d = nc.dram_tensor("wht", [128, HC, G3], WDT, kind="ExternalInput")
    g_d = nc.dram_tensor("g", [VOCAB, G3], WDT, kind="ExternalInput")
    decw_d = nc.dram_tensor("decwt", [128, HC], WDT, kind="ExternalInput")
    decb_d = nc.dram_tensor("decb", [1, 1], F32, kind="ExternalInput")

    n_win = T_steps // DEC_W
    hlast_d = nc.dram_tensor("hlast", [128, HC, BL], WDT, kind="ExternalOutput")
    deco_d = nc.dram_tensor("deco", [n_win, DEC_W * BL], F32, kind="ExternalOutput")

    from contextlib import ExitStack
    with tile.TileContext(nc) as tc, ExitStack() as ctx:
        cpool = ctx.enter_context(tc.tile_pool(name="const", bufs=1))
        spool = ctx.enter_context(tc.tile_pool(name="work", bufs=2))
        hpool = ctx.enter_context(tc.tile_pool(name="hstate", bufs=3))
        ppool = ctx.enter_context(tc.tile_pool(name="psum", bufs=2, space="PSUM"))

        h_cur = hpool.tile([128, HC, BL], F32, tag="h")
        nc.sync.dma_start(h_cur[:], h0_d.ap())
        g_sb = cpool.tile([VOCAB, G3], WDT)
        nc.scalar.dma_start(g_sb[:], g_d.ap())
        wht_sb = cpool.tile([128, HC, G3], WDT)
        nc.scalar.dma_start(wht_sb[:], wht_d.ap())
        oh_sb = cpool.tile([VOCAB, T_steps * BL], WDT)
        n_oh_chunks = max(1, T_steps // 64)
        ohc = T_steps * BL // n_oh_chunks
        for i in range(n_oh_chunks):
            nc.sync.dma_start(oh_sb[:, i * ohc:(i + 1) * ohc],
                              oh_d.ap()[:, i * ohc:(i + 1) * ohc])
        decw_sb = cpool.tile([128, HC], WDT)
        nc.scalar.dma_start(decw_sb[:], decw_d.ap())
        decb_sb = cpool.tile([1, 1], F32)
        nc.scalar.dma_start(decb_sb[:], decb_d.ap())
        # relu(h_t) history, [128, t, kc, b]
        hist = cpool.tile([128, T_steps, HC, BL], WDT)

        h_w = hpool.tile([128, HC, BL], WDT, tag="hw")
        nc.vector.tensor_copy(h_w.rearrange("p a b -> p (a b)"),
                              h_cur.rearrange("p a b -> p (a b)"))

        # psum tiles are allocated one step ahead so the h-independent
        # one-hot (gi) matmuls of step t+1 can run in step t's chain shadow.
        def alloc_step_tiles():
            p_r = ppool.tile([128, 2 * BL], F32, tag="r", name="p_r")
            p_z = ppool.tile([128, 2 * BL], F32, tag="z", name="p_z")
            p_n = ppool.tile([128, 2, BL, 2], F32, tag="nn", name="p_n")
            return p_r, p_z, p_n

        def emit_gi(tiles, t):
            # start=True only on the FIRST matmul into each psum bank per
            # step-cycle: start clears has_written for the WHOLE bank, so a
            # second start=True would wipe the first region's accumulate bits.
            # Later region-initial matmuls use start=False and overwrite
            # (bits are clear there), while same-region matmuls accumulate.
            p_r, p_z, p_n = tiles
            oh_t = oh_sb[:, t * BL:(t + 1) * BL]            # [40, 32]
            for c in range(2):      # r chunks
                nc.tensor.matmul(p_r[:, c * BL:(c + 1) * BL],
                                 g_sb[:, c * 128:(c + 1) * 128],
                                 oh_t, start=(c == 0), stop=False,
                                 skip_group_check=True)
            for c in range(2):      # z chunks
                nc.tensor.matmul(p_z[:, c * BL:(c + 1) * BL],
                                 g_sb[:, (2 + c) * 128:(3 + c) * 128],
                                 oh_t, start=(c == 0), stop=False,
                                 skip_group_check=True)
            last = None
            for c in range(2):      # n chunks (gi-only, odd interleave slots)
                last = nc.tensor.matmul(p_n[:, c, :, 1],
                                        g_sb[:, (4 + c) * 128:(5 + c) * 128],
                                        oh_t, start=(c == 0), stop=False,
                                        skip_group_check=True)
            return last

        d0 = cpool.tile([128, 2, BL, 2], WDT)   # [0 | r] interleave for scan 1
        nc.vector.memset(d0[:], 0.0)

        anchor_pe = {}   # step -> last gi matmul inst (PE ordering anchor)
        anchor_act = {}  # step -> tanh inst (ACT ordering anchor)

        def emit_decode(w):
            # out[t,b] = sigmoid(sum_h relu(h)[h,t,b] * dec_w[h] + b)
            # Ordering-only deps pin each piece into a specific step's engine
            # slack so the static scheduler cannot hoist it into the critical
            # FIFO position of an earlier step.
            base = min(w * DEC_W + DEC_W, T_steps - 1)
            HW2 = DEC_W // 2
            pd = ppool.tile([1, DEC_W * BL], F32, tag="dec", name="pd", bufs=1)
            k = 0
            for half in range(2):
                t0, t1 = w * DEC_W + half * HW2, w * DEC_W + (half + 1) * HW2
                dst = pd[:, half * HW2 * BL:(half + 1) * HW2 * BL]
                for kc in range(HC):
                    rhs = hist[:, t0:t1, kc, :]
                    mm = nc.tensor.matmul(dst, decw_sb[:, kc:kc + 1], rhs,
                                          start=(kc == 0), stop=(kc == HC - 1))
                    a = anchor_pe.get(min(base + k, T_steps - 1))
                    if a is not None:
                        add_dep_helper(mm.ins, a.ins, sync=False,
                                       reason="decode mm in PE slack")
                    k += 1
            ds = spool.tile([1, DEC_W * BL], F32, tag="ds")
            for half in range(2):
                sl = slice(half * HW2 * BL, (half + 1) * HW2 * BL)
                sg = nc.scalar.activation(ds[:, sl], pd[:, sl], AF.Sigmoid,
                                          bias=decb_sb[:, 0:1])
                a = anchor_act.get(min(base + 4 + half, T_steps - 1))
                if a is not None:
                    add_dep_helper(sg.ins, a.ins, sync=False,
                                   reason="decode sigmoid after tanh")
            nc.sync.dma_start(deco_d.ap()[w:w + 1, :], ds[:])

        cur_tiles = alloc_step_tiles()
        emit_gi(cur_tiles, 0)

        for t in range(T_steps):
            p_r, p_z, p_n = cur_tiles

            # gh accumulation: r chunks first (gate the chain), then z, then hn
            for c in range(2):
                dst = p_r[:, c * BL:(c + 1) * BL]
                nc.tensor.matmul(dst, wht_sb[:, 0, c * 128:(c + 1) * 128],
                                 h_w[:, 0, :], start=False, stop=False,
                                 skip_group_check=True)
                nc.tensor.matmul(dst, wht_sb[:, 1, c * 128:(c + 1) * 128],
                                 h_w[:, 1, :], start=False, stop=(c == 1),
                                 skip_group_check=True)
            for c in range(2):
                dst = p_z[:, c * BL:(c + 1) * BL]
                nc.tensor.matmul(dst, wht_sb[:, 0, (2 + c) * 128:(3 + c) * 128],
                                 h_w[:, 0, :], start=False, stop=False,
                                 skip_group_check=True)
                nc.tensor.matmul(dst, wht_sb[:, 1, (2 + c) * 128:(3 + c) * 128],
                                 h_w[:, 1, :], start=False, stop=(c == 1),
                                 skip_group_check=True)
            for c in range(2):      # hn: gh only (bank bits cleared by gi_n0)
                dst = p_n[:, c, :, 0]
                nc.tensor.matmul(dst, wht_sb[:, 0, (4 + c) * 128:(5 + c) * 128],
                                 h_w[:, 0, :], start=False, stop=False,
                                 skip_group_check=True)
                nc.tensor.matmul(dst, wht_sb[:, 1, (4 + c) * 128:(5 + c) * 128],
                                 h_w[:, 1, :], start=False, stop=(c == 1),
                                 skip_group_check=True)

            # sigmoid split: r first (unblocks the scan), z next.
            # r lands strided into d0's odd slots for the fused scan.
            nc.scalar.activation(d0[:, :, :, 1].rearrange("p c b -> p (c b)"),
                                 p_r[:], AF.Sigmoid)
            sz = spool.tile([128, 2 * BL], WDT, tag="sz")
            nc.scalar.activation(sz[:], p_z[:], AF.Sigmoid)

            # fused r*hn + gn via tensor_tensor_scan over interleaved pairs:
            # d0 = [0 | r], d1 = [hn | gn]  ->  odd positions = r*hn + gn
            nscr = spool.tile([128, 2, BL, 2], WDT, tag="nscr")
            i_tts = nc.vector.tensor_tensor_scan(
                nscr.rearrange("p c b w -> p (c b w)"),
                d0[:].rearrange("p c b w -> p (c b w)"),
                p_n[:].rearrange("p c b w -> p (c b w)"),
                0.0, OP.mult, OP.add)

            # tail operands, both in tanh's shadow:
            sm = spool.tile([128, 2 * BL], WDT, tag="sm")     # 1 - z
            nc.vector.tensor_scalar(sm[:], sz[:], 1.0, -1.0,
                                    OP.subtract, OP.mult)
            hw_flat = h_w.rearrange("p a b -> p (a b)")
            zh = spool.tile([128, 2 * BL], WDT, tag="zh")     # z * h
            nc.vector.tensor_tensor(zh[:], sz[:], hw_flat, OP.mult)

            n_t = spool.tile([128, 2 * BL], WDT, tag="nt")
            anchor_act[t] = nc.scalar.activation(
                n_t[:], nscr[:, :, :, 1].rearrange("p c b -> p (c b)"), AF.Tanh)
            vp = spool.tile([128, 2 * BL], WDT, tag="vp")     # (1-z)*n
            nc.vector.tensor_tensor(vp[:], sm[:], n_t[:], OP.mult)
            hw_new = hpool.tile([128, HC, BL], WDT, tag="hw")
            nc.vector.tensor_tensor(hw_new.rearrange("p a b -> p (a b)"),
                                    zh[:], vp[:], OP.add)
            h_w = hw_new
            # prefetch next step's gi matmuls into this chain's shadow
            if t + 1 < T_steps:
                cur_tiles = alloc_step_tiles()
                anchor_pe[t] = emit_gi(cur_tiles, t + 1)
            # relu'd history for the decoder (DVE slack, off the critical path)
            nc.vector.tensor_scalar_max(hist[:, t, :, :].rearrange("p a b -> p (a b)"),
                                        h_w.rearrange("p a b -> p (a b)"), 0.0)

        nc.sync.dma_start(hlast_d.ap(), h_w[:])

        # decoder post-pass
        for w in range(n_win):
            emit_decode(w)

    nc.compile()
    return nc


def _prep_inputs(x, h0, embed, w_ih, w_hh, dec_w, dec_b, T_steps=T):
    x = np.asarray(x)
    h0 = np.asarray(h0, dtype=np.float32)
    embed = np.asarray(embed, dtype=np.float32)
    w_ih = np.asarray(w_ih, dtype=np.float32)
    w_hh = np.asarray(w_hh, dtype=np.float32)
    dec_w = np.asarray(dec_w, dtype=np.float32)
    dec_b = np.asarray(dec_b, dtype=np.float32)

    G = (embed @ w_ih.T).astype(WNP)                                   # [40, 768]
    wht = np.ascontiguousarray(
        w_hh.T.reshape(HC, 128, G3).transpose(1, 0, 2)).astype(WNP)    # [128,2,768]
    decwt = np.ascontiguousarray(dec_w.reshape(HC, 128).T).astype(WNP)  # [128, 2]
    decb = dec_b.reshape(1, 1)

    in_maps = []
    for c in range(NCORES):
        xc = x[c * BL:(c + 1) * BL, :T_steps]                          # [32, T]
        idx = np.ascontiguousarray(xc.T).reshape(-1)                   # t-major
        oh = (np.arange(VOCAB)[:, None] == idx[None, :]).astype(WNP)
        h0c = h0[0, c * BL:(c + 1) * BL, :]                            # [32, 256]
        h0t = np.ascontiguousarray(
            h0c.T.reshape(HC, 128, BL).transpose(1, 0, 2))             # [128,2,32]
        in_maps.append({
            "oh": oh, "h0t": h0t, "wht": wht, "g": G,
            "decwt": decwt, "decb": decb,
        })
    return in_maps


_NC_CACHE = {}


def kernel(x, h0, embed, w_ih, w_hh, dec_w, dec_b, T_steps=T, want_trace=False):
    if T_steps not in _NC_CACHE:
        _NC_CACHE[T_steps] = _build_bass(T_steps)
    nc = _NC_CACHE[T_steps]
    in_maps = _prep_inputs(x, h0, embed, w_ih, w_hh, dec_w, dec_b, T_steps)
    res = run_bass_kernel_spmd(nc, in_maps, core_ids=list(range(NCORES)),
                               trace=want_trace)

    n_win = T_steps // DEC_W
    outs, hlasts = [], []
    for c in range(NCORES):
        r = res.results[c]
        deco = r["deco"].reshape(n_win, DEC_W, BL).transpose(2, 0, 1)
        outs.append(deco.reshape(BL, T_steps))
        hl = r["hlast"].transpose(2, 1, 0).reshape(BL, H)
        hlasts.append(hl)
    out = np.concatenate(outs, axis=0)[:, :, None].astype(np.float32)  # [B,T,1]
    h_last = np.concatenate(hlasts, axis=0)[None].astype(np.float32)   # [1,B,H]
    if want_trace:
        return (out, h_last), res
    return out, h_last


# revision 40
# speedup vs baseline: 1.2629x; 1.0001x over previous
"""Trainium2 Bass kernel for nn_GRUNetBinaryEmbeding.

Reference computation (PyTorch GRU semantics, gate order r,z,n; no biases):
    emb = embed[x]                                # [B,T,H]
    per t: gi = emb_t @ w_ih.T ; gh = h @ w_hh.T
           r = sig(gi_r + gh_r); z = sig(gi_z + gh_z)
           n = tanh(gi_n + r * gh_n)
           h = (1-z)*n + z*h
    out  = sigmoid(relu(h_t) @ dec_w.T + dec_b)   # [B,T,1]
    also returns h_last [1,B,H]

Shapes: VOCAB=40, H=256, OUT=1, B=256, T=512.

Strategy (8 NeuronCores, data-parallel over batch, B_local=32/core):
  - key algebraic move: gi = (embed @ w_ih.T)[x] = G[x]; realized on device as
    one-hot(x) matmuls against the precomputed G table [40,768], which
    accumulate directly into the same PSUM tiles as the h @ w_hh.T partial
    products (PE does the gi+gh adds for the r,z gates for free).
  - transposed layout everywhere: [hidden -> partitions, batch -> free], so
    elementwise gate math runs on full 128-lane tiles [128, 64] and the
    recurrent matmuls need no per-step transpose (rhs = h.T chunks).
  - per step: 18 matmuls (6 gate M-chunks x (2 h K-chunks + one-hot), N=32)
    into 2 PSUM banks; sigmoid/tanh on ScalarE; 6 DVE ops for the cell update.
  - decoder (dec_w [1,256]) runs as a post-pass over the relu'd hidden-state
    history kept in SBUF: M=1 matmuls + fused sigmoid(+bias) on ScalarE.
"""

import numpy as np
import ml_dtypes

import concourse.bass as bass
import concourse.tile as tile
from concourse.tile import add_dep_helper
from concourse import bacc
from concourse import mybir
from concourse.bass_utils import run_bass_kernel_spmd

# ---- problem constants (hardcoded per the task contract) ----
VOCAB, H, B, T = 40, 256, 256, 512
NCORES = 8
BL = B // NCORES          # batch per core = 32
HC = 2                    # hidden chunks of 128
G3 = 3 * H                # 768 gate dim
DEC_W = 16                # decoder window (timesteps per decode matmul)

# weight/activation dtype for matmul operands and carried state (PSUM and all
# gate math accumulate in fp32 on-engine; bf16 keeps DVE 2x modes and FWL).
WNP = ml_dtypes.bfloat16
WDT = mybir.dt.bfloat16
F32 = mybir.dt.float32
AF = mybir.ActivationFunctionType
OP = mybir.AluOpType


def _build_bass(T_steps: int = T):
    nc = bacc.Bacc("TRN2", target_bir_lowering=False, debug=False)

    # ---- DRAM I/O ----
    oh_d = nc.dram_tensor("oh", [VOCAB, T_steps * BL], WDT, kind="ExternalInput")
    h0_d = nc.dram_tensor("h0t", [128, HC, BL], F32, kind="ExternalInput")
    wht_d = nc.dram_tensor("wht", [128, HC, G3], WDT, kind="ExternalInput")
    g_d = nc.dram_tensor("g", [VOCAB, G3], WDT, kind="ExternalInput")
    decw_d = nc.dram_tensor("decwt", [128, HC], WDT, kind="ExternalInput")
    decb_d = nc.dram_tensor("decb", [1, 1], F32, kind="ExternalInput")

    n_win = T_steps // DEC_W
    hlast_d = nc.dram_tensor("hlast", [128, HC, BL], WDT, kind="ExternalOutput")
    deco_d = nc.dram_tensor("deco", [n_win, DEC_W * BL], F32, kind="ExternalOutput")

    from contextlib import ExitStack
    with tile.TileContext(nc) as tc, ExitStack() as ctx:
        cpool = ctx.enter_context(tc.tile_pool(name="const", bufs=1))
        spool = ctx.enter_context(tc.tile_pool(name="work", bufs=3))
        hpool = ctx.enter_context(tc.tile_pool(name="hstate", bufs=3))
        ppool = ctx.enter_context(tc.tile_pool(name="psum", bufs=2, space="PSUM"))

        h_cur = hpool.tile([128, HC, BL], F32, tag="h")
        nc.sync.dma_start(h_cur[:], h0_d.ap())
        g_sb = cpool.tile([VOCAB, G3], WDT)
        nc.scalar.dma_start(g_sb[:], g_d.ap())
        wht_sb = cpool.tile([128, HC, G3], WDT)
        nc.scalar.dma_start(wht_sb[:], wht_d.ap())
        oh_sb = cpool.tile([VOCAB, T_steps * BL], WDT)
        n_oh_chunks = max(1, T_steps // 64)
        ohc = T_steps * BL // n_oh_chunks
        for i in range(n_oh_chunks):
            nc.sync.dma_start(oh_sb[:, i * ohc:(i + 1) * ohc],
                              oh_d.ap()[:, i * ohc:(i + 1) * ohc])
        decw_sb = cpool.tile([128, HC], WDT)
        nc.scalar.dma_start(decw_sb[:], decw_d.ap())
        decb_sb = cpool.tile([1, 1], F32)
        nc.scalar.dma_start(decb_sb[:], decb_d.ap())
        # relu(h_t) history, [128, t, kc, b]
        hist = cpool.tile([128, T_steps, HC, BL], WDT)

        h_w = hpool.tile([128, HC, BL], WDT, tag="hw")
        nc.vector.tensor_copy(h_w.rearrange("p a b -> p (a b)"),
                              h_cur.rearrange("p a b -> p (a b)"))

        # psum tiles are allocated one step ahead so the h-independent
        # one-hot (gi) matmuls of step t+1 can run in step t's chain shadow.
        def alloc_step_tiles():
            p_r = ppool.tile([128, 2 * BL], F32, tag="r", name="p_r")
            p_z = ppool.tile([128, 2 * BL], F32, tag="z", name="p_z")
            p_n = ppool.tile([128, 2, BL, 2], F32, tag="nn", name="p_n")
            return p_r, p_z, p_n

        def emit_gi(tiles, t):
            # start=True only on the FIRST matmul into each psum bank per
            # step-cycle: start clears has_written for the WHOLE bank, so a
            # second start=True would wipe the first region's accumulate bits.
            # Later region-initial matmuls use start=False and overwrite
            # (bits are clear there), while same-region matmuls accumulate.
            p_r, p_z, p_n = tiles
            oh_t = oh_sb[:, t * BL:(t + 1) * BL]            # [40, 32]
            for c in range(2):      # r chunks
                nc.tensor.matmul(p_r[:, c * BL:(c + 1) * BL],
                                 g_sb[:, c * 128:(c + 1) * 128],
                                 oh_t, start=(c == 0), stop=False,
                                 skip_group_check=True)
            for c in range(2):      # z chunks
                nc.tensor.matmul(p_z[:, c * BL:(c + 1) * BL],
                                 g_sb[:, (2 + c) * 128:(3 + c) * 128],
                                 oh_t, start=(c == 0), stop=False,
                                 skip_group_check=True)
            last = None
            for c in range(2):      # n chunks (gi-only, odd interleave slots)
                last = nc.tensor.matmul(p_n[:, c, :, 1],
                                        g_sb[:, (4 + c) * 128:(5 + c) * 128],
                                        oh_t, start=(c == 0), stop=False,
                                        skip_group_check=True)
            return last

        d0 = cpool.tile([128, 2, BL, 2], WDT)   # [0 | r] interleave for scan 1
        nc.vector.memset(d0[:], 0.0)

        anchor_pe = {}   # step -> last gi matmul inst (PE ordering anchor)
        anchor_act = {}  # step -> tanh inst (ACT ordering anchor)

        def emit_decode(w):
            # out[t,b] = sigmoid(sum_h relu(h)[h,t,b] * dec_w[h] + b)
            # Ordering-only deps pin each piece into a specific step's engine
            # slack so the static scheduler cannot hoist it into the critical
            # FIFO position of an earlier step.
            base = min(w * DEC_W + DEC_W, T_steps - 1)
            HW2 = DEC_W // 2
            pd = ppool.tile([1, DEC_W * BL], F32, tag="dec", name="pd", bufs=1)
            k = 0
            for half in range(2):
                t0, t1 = w * DEC_W + half * HW2, w * DEC_W + (half + 1) * HW2
                dst = pd[:, half * HW2 * BL:(half + 1) * HW2 * BL]
                for kc in range(HC):
                    rhs = hist[:, t0:t1, kc, :]
                    mm = nc.tensor.matmul(dst, decw_sb[:, kc:kc + 1], rhs,
                                          start=(kc == 0), stop=(kc == HC - 1))
                    a = anchor_pe.get(min(base + k, T_steps - 1))
                    if a is not None:
                        add_dep_helper(mm.ins, a.ins, sync=False,
                                       reason="decode mm in PE slack")
                    k += 1
            ds = spool.tile([1, DEC_W * BL], F32, tag="ds")
            for half in range(2):
                sl = slice(half * HW2 * BL, (half + 1) * HW2 * BL)
                sg = nc.scalar.activation(ds[:, sl], pd[:, sl], AF.Sigmoid,
                                          bias=decb_sb[:, 0:1])
                a = anchor_act.get(min(base + 4 + half, T_steps - 1))
                if a is not None:
                    add_dep_helper(sg.ins, a.ins, sync=False,
                                   reason="decode sigmoid after tanh")
            nc.sync.dma_start(deco_d.ap()[w:w + 1, :], ds[:])

        cur_tiles = alloc_step_tiles()
        emit_gi(cur_tiles, 0)

        for t in range(T_steps):
            p_r, p_z, p_n = cur_tiles

            # gh accumulation: r chunks first (gate the chain), then z, then hn
            for c in range(2):
                dst = p_r[:, c * BL:(c + 1) * BL]
                nc.tensor.matmul(dst, wht_sb[:, 0, c * 128:(c + 1) * 128],
                                 h_w[:, 0, :], start=False, stop=False,
                                 skip_group_check=True)
                nc.tensor.matmul(dst, wht_sb[:, 1, c * 128:(c + 1) * 128],
                                 h_w[:, 1, :], start=False, stop=(c == 1),
                                 skip_group_check=True)
            for c in range(2):
                dst = p_z[:, c * BL:(c + 1) * BL]
                nc.tensor.matmul(dst, wht_sb[:, 0, (2 + c) * 128:(3 + c) * 128],
                                 h_w[:, 0, :], start=False, stop=False,
                                 skip_group_check=True)
                nc.tensor.matmul(dst, wht_sb[:, 1, (2 + c) * 128:(3 + c) * 128],
                                 h_w[:, 1, :], start=False, stop=(c == 1),
                                 skip_group_check=True)
            for c in range(2):      # hn: gh only (bank bits cleared by gi_n0)
                dst = p_n[:, c, :, 0]
                nc.tensor.matmul(dst, wht_sb[:, 0, (4 + c) * 128:(5 + c) * 128],
                                 h_w[:, 0, :], start=False, stop=False,
                                 skip_group_check=True)
                nc.tensor.matmul(dst, wht_sb[:, 1, (4 + c) * 128:(5 + c) * 128],
                                 h_w[:, 1, :], start=False, stop=(c == 1),
                                 skip_group_check=True)

            # sigmoid split: r first (unblocks the scan), z next.
            # r lands strided into d0's odd slots for the fused scan.
            nc.scalar.activation(d0[:, :, :, 1].rearrange("p c b -> p (c b)"),
                                 p_r[:], AF.Sigmoid)
            sz = spool.tile([128, 2 * BL], WDT, tag="sz")
            nc.scalar.activation(sz[:], p_z[:], AF.Sigmoid)

            # fused r*hn + gn via tensor_tensor_scan over interleaved pairs:
            # d0 = [0 | r], d1 = [hn | gn]  ->  odd positions = r*hn + gn
            nscr = spool.tile([128, 2, BL, 2], WDT, tag="nscr")
            i_tts = nc.vector.tensor_tensor_scan(
                nscr.rearrange("p c b w -> p (c b w)"),
                d0[:].rearrange("p c b w -> p (c b w)"),
                p_n[:].rearrange("p c b w -> p (c b w)"),
                0.0, OP.mult, OP.add)

            # tail operands, both in tanh's shadow:
            sm = spool.tile([128, 2 * BL], WDT, tag="sm")     # 1 - z
            nc.vector.tensor_scalar(sm[:], sz[:], 1.0, -1.0,
                                    OP.subtract, OP.mult)
            hw_flat = h_w.rearrange("p a b -> p (a b)")
            zh = spool.tile([128, 2 * BL], WDT, tag="zh")     # z * h
            nc.vector.tensor_tensor(zh[:], sz[:], hw_flat, OP.mult)

            n_t = spool.tile([128, 2 * BL], WDT, tag="nt")
            anchor_act[t] = nc.scalar.activation(
                n_t[:], nscr[:, :, :, 1].rearrange("p c b -> p (c b)"), AF.Tanh)
            vp = spool.tile([128, 2 * BL], WDT, tag="vp")     # (1-z)*n
            nc.vector.tensor_tensor(vp[:], sm[:], n_t[:], OP.mult)
            hw_new = hpool.tile([128, HC, BL], WDT, tag="hw")
            nc.vector.tensor_tensor(hw_new.rearrange("p a b -> p (a b)"),
                                    zh[:], vp[:], OP.add)
            h_w = hw_new
            # prefetch next step's gi matmuls into this chain's shadow
            if t + 1 < T_steps:
                cur_tiles = alloc_step_tiles()
                anchor_pe[t] = emit_gi(cur_tiles, t + 1)
            # relu'd history for the decoder (DVE slack, off the critical path)
            nc.vector.tensor_scalar_max(hist[:, t, :, :].rearrange("p a b -> p (a b)"),
                                        h_w.rearrange("p a b -> p (a b)"), 0.0)

        nc.sync.dma_start(hlast_d.ap(), h_w[:])

        # decoder post-pass
        for w in range(n_win):
            emit_decode(w)

    nc.compile()
    return nc


def _prep_inputs(x, h0, embed, w_ih, w_hh, dec_w, dec_b, T_steps=T):
    x = np.asarray(x)
    h0 = np.asarray(h0, dtype=np.float32)
    embed = np.asarray(embed, dtype=np.float32)
    w_ih = np.asarray(w_ih, dtype=np.float32)
    w_hh = np.asarray(w_hh, dtype=np.float32)
    dec_w = np.asarray(dec_w, dtype=np.float32)
    dec_b = np.asarray(dec_b, dtype=np.float32)

    G = (embed @ w_ih.T).astype(WNP)                                   # [40, 768]
    wht = np.ascontiguousarray(
        w_hh.T.reshape(HC, 128, G3).transpose(1, 0, 2)).astype(WNP)    # [128,2,768]
    decwt = np.ascontiguousarray(dec_w.reshape(HC, 128).T).astype(WNP)  # [128, 2]
    decb = dec_b.reshape(1, 1)

    in_maps = []
    for c in range(NCORES):
        xc = x[c * BL:(c + 1) * BL, :T_steps]                          # [32, T]
        idx = np.ascontiguousarray(xc.T).reshape(-1)                   # t-major
        oh = (np.arange(VOCAB)[:, None] == idx[None, :]).astype(WNP)
        h0c = h0[0, c * BL:(c + 1) * BL, :]                            # [32, 256]
        h0t = np.ascontiguousarray(
            h0c.T.reshape(HC, 128, BL).transpose(1, 0, 2))             # [128,2,32]
        in_maps.append({
            "oh": oh, "h0t": h0t, "wht": wht, "g": G,
            "decwt": decwt, "decb": decb,
        })
    return in_maps


_NC_CACHE = {}


def kernel(x, h0, embed, w_ih, w_hh, dec_w, dec_b, T_steps=T, want_trace=False):
    if T_steps not in _NC_CACHE:
        _NC_CACHE[T_steps] = _build_bass(T_steps)
    nc = _NC_CACHE[T_steps]
    in_maps = _prep_inputs(x, h0, embed, w_ih, w_hh, dec_w, dec_b, T_steps)
    res = run_bass_kernel_spmd(nc, in_maps, core_ids=list(range(NCORES)),
                               trace=want_trace)

    n_win = T_steps // DEC_W
    outs, hlasts = [], []
    for c in range(NCORES):
        r = res.results[c]
        deco = r["deco"].reshape(n_win, DEC_W, BL).transpose(2, 0, 1)
        outs.append(deco.reshape(BL, T_steps))
        hl = r["hlast"].transpose(2, 1, 0).reshape(BL, H)
        hlasts.append(hl)
    out = np.concatenate(outs, axis=0)[:, :, None].astype(np.float32)  # [B,T,1]
    h_last = np.concatenate(hlasts, axis=0)[None].astype(np.float32)   # [1,B,H]
    if want_trace:
        return (out, h_last), res
    return out, h_last
